# revision 28
# baseline (speedup 1.0000x reference)
"""Trainium2 Bass kernel for nn_Convolution (gnn_message_passing).

Strategy (no collectives needed):
  - Edges are sorted by destination node and partitioned across the 8 cores
    by dst range (each core owns N/8 destination nodes). Node features and
    weights are replicated; each core redundantly computes the lin1 table
    for all nodes (phase 1), then processes only edges destined to its own
    node slice (phase 2) and writes its slice of the output.
  - Phase 2 runs in "windows" of 128 destination slots. Per window:
    gather y=l[src] rows with dma_gather, radial MLP on PE, message build on
    DVE (bf16), segment-sum via selection-matrix matmuls accumulating in
    PSUM, then one fused (lin2 + self-interaction) matmul chain.
  - All matmuls are bf16 (fp32 LOW_HIGH matmuls are ~10x slower on PE).
  - All e3nn normalization constants and node_attr are folded into weights /
    edge attributes on the host.
"""

import sys

for _p in ("/opt/trn_rl_repo",):
    if _p not in sys.path:
        sys.path.insert(0, _p)

import numpy as np
import ml_dtypes

import concourse.bass as bass
import concourse.bacc as bacc
import concourse.mybir as mybir
import concourse.tile as tile
from concourse import bass_utils

BF16 = ml_dtypes.bfloat16

# Problem constants (hardcoded per contract)
N_NODES = 50000
N_EDGES = 800000
MUL0, MUL1 = 64, 32
N_BASIS, N_RADIAL = 10, 100
NUM_NEIGHBORS = 16.0
INV_SQRT3 = np.float32(1.0 / np.sqrt(3.0))
RELU_GAIN = np.float32(np.sqrt(2.0))
FAN_L2 = np.float32(np.sqrt(MUL0 + MUL1))

N_CORES = 8
SPLIT = 32768          # dma_gather idx is int16 -> split src tables
LROW = 256             # l-table row elems (bf16) -> 512B rows (256B-aligned)
WIN = 128              # dst slots per window

_LAST_RESULTS = None   # BassKernelResults of the most recent run (for test.py)


# --------------------------------------------------------------------------
# Device program
# --------------------------------------------------------------------------

def build_program(n_nodes, npc, n_win, t_a, t_b, num_cores, split=SPLIT):
    """Build the SPMD Bass program. npc = nodes per core."""
    tt = t_a + t_b
    ew = tt * 128            # padded edges per window
    e_core = n_win * ew
    # meta: cntA/cntB (2xi32) + ea(tt*4) + dl(tt) + idxA(t_a*8) + idxB(t_b*8)
    mcols = ((4 + tt * 13) + 1) // 2 * 2
    oE = 4
    oD = 4 + tt * 4
    oA = 4 + tt * 5
    oB = 4 + tt * 5 + t_a * 8
    f32, bf16, i16 = mybir.dt.float32, mybir.dt.bfloat16, mybir.dt.int16
    i32 = mybir.dt.int32

    nc = bacc.Bacc("TRN2", target_bir_lowering=False, debug=False,
                   enable_asserts=False, num_devices=num_cores)

    # DRAM I/O (per-core data; weights replicated across cores)
    xaT = nc.dram_tensor("xaT", [160, n_nodes], bf16, kind="ExternalInput").ap()
    xwin = nc.dram_tensor("xwin", [160, n_win * 128], bf16, kind="ExternalInput").ap()
    embT = nc.dram_tensor("embT", [10, e_core], bf16, kind="ExternalInput").ap()
    meta = nc.dram_tensor("meta", [128, n_win * mcols], i16, kind="ExternalInput").ap()
    Wbd = nc.dram_tensor("Wbd", [128, 160], bf16, kind="ExternalInput").ap()
    W2b = nc.dram_tensor("W2b", [32, 160], bf16, kind="ExternalInput").ap()
    Wfc1 = nc.dram_tensor("Wfc1", [10, 100], bf16, kind="ExternalInput").ap()
    Wfc2b = nc.dram_tensor("Wfc2b", [100, 192], bf16, kind="ExternalInput").ap()
    Wbig = nc.dram_tensor("Wbig", [128, 5 * 160], bf16, kind="ExternalInput").ap()
    Asel = nc.dram_tensor("Asel", [128, n_win * tt * 128], bf16,
                          kind="ExternalInput").ap()
    out = nc.dram_tensor("out", [n_win * 128, 160], f32, kind="ExternalOutput").ap()

    mult = mybir.AluOpType.mult
    addop = mybir.AluOpType.add
    iseq = mybir.AluOpType.is_equal
    relu = mybir.ActivationFunctionType.Relu

    with tile.TileContext(nc) as tc:
        with (
            tc.tile_pool(name="const", bufs=1) as cpool,
            tc.tile_pool(name="ld", bufs=2) as ldpool,
            tc.tile_pool(name="ltab", bufs=1, space="DRAM") as dpool,
            tc.tile_pool(name="win", bufs=4) as wpool,
            tc.tile_pool(name="scr", bufs=2) as spool,
        ):
            # ---- constants to SBUF
            wbd_sb = cpool.tile([128, 160], bf16)
            nc.sync.dma_start(out=wbd_sb[:], in_=Wbd)
            w2b_sb = cpool.tile([32, 160], bf16)
            nc.sync.dma_start(out=w2b_sb[:], in_=W2b)
            wfc1_sb = cpool.tile([10, 100], bf16)
            nc.sync.dma_start(out=wfc1_sb[:], in_=Wfc1)
            wfc2_sb = cpool.tile([100, 192], bf16)
            nc.sync.dma_start(out=wfc2_sb[:], in_=Wfc2b)
            wbig_sb = cpool.tile([128, 5 * 160], bf16)
            nc.sync.dma_start(out=wbig_sb[:], in_=Wbig)

            ltabA = dpool.tile([split, LROW], bf16)
            ltabB = dpool.tile([n_nodes - split, LROW], bf16)

            # ---- phase 1: l table (lin1 of all nodes), bf16 rows in DRAM
            # 2 node-tiles per PSUM bank; 2 fused block-diagonal matmuls per
            # tile; table split A/B so A-gathers can start before B is done.
            CH = 4096
            lps_ctx = tc.tile_pool(name="lps", bufs=4, space="PSUM")
            lpsum = lps_ctx.__enter__()
            for c0 in range(0, n_nodes, CH):
                cw = min(CH, n_nodes - c0)
                xa = ldpool.tile([128, CH], bf16, tag="xa", bufs=3)
                nc.sync.dma_start(out=xa[:, :cw], in_=xaT[0:128, c0:c0 + cw])
                xb = ldpool.tile([32, CH], bf16, tag="xb", bufs=3)
                nc.sync.dma_start(out=xb[:, :cw], in_=xaT[128:160, c0:c0 + cw])
                for t0 in range(0, cw, 384):
                    bw = min(384, cw - t0)
                    nb = (bw + 127) // 128
                    pl = lpsum.tile([128, 480], f32, tag="pl")
                    for b in range(nb):
                        nn_ = min(128, bw - b * 128)
                        nc.tensor.matmul(out=pl[:nn_, b * 160:b * 160 + 160],
                                         lhsT=xa[:, t0 + b * 128:t0 + b * 128 + nn_],
                                         rhs=wbd_sb[:],
                                         start=True, stop=False)
                        nc.tensor.matmul(out=pl[:nn_, b * 160:b * 160 + 160],
                                         lhsT=xb[:, t0 + b * 128:t0 + b * 128 + nn_],
                                         rhs=w2b_sb[:],
                                         start=False, stop=True)
                    lt = ldpool.tile([128, 480], bf16, tag="lt", bufs=4)
                    nc.scalar.copy(out=lt[:, :nb * 160], in_=pl[:, :nb * 160])
                    r0 = c0 + t0
                    tgt, ro = (ltabA, r0) if r0 < split else (ltabB, r0 - split)
                    if bw % 128 == 0:
                        nc.sync.dma_start(
                            out=tgt[ro:ro + bw, 0:160]
                                .rearrange("(b p) e -> p b e", p=128),
                            in_=lt[:].rearrange("p (b e) -> p b e", e=160)[:, :nb, :])
                    else:
                        nc.sync.dma_start(out=tgt[ro:ro + bw, 0:160],
                                          in_=lt[:bw, 0:160])
            lps_ctx.__exit__(None, None, None)

            # ---- phase 2: windows
            ps_ctx = tc.tile_pool(name="ps", bufs=2, space="PSUM")
            psum = ps_ctx.__enter__()
            n5 = (tt * 128 + 511) // 512
            # explicit rotation (4-deep) instead of a pool so the buffers can
            # be zeroed once up front (keeps padding rows finite).
            y_bufs = [cpool.tile([128, tt * 256], bf16, name=f"ybuf{i}")
                      for i in range(4)]
            for yb in y_bufs:
                nc.vector.memset(yb[:], 0.0)
            for w in range(n_win):
                # loads (meta packs ea/dl/idxA/idxB in one i16 tensor)
                meta_w = wpool.tile([128, mcols], i16, tag="meta")
                nc.sync.dma_start(out=meta_w[:], in_=meta[:, w * mcols:(w + 1) * mcols])
                ea_w = meta_w[:, oE:oE + tt * 4].bitcast(bf16)
                y_w = y_bufs[w % 4]
                nc.gpsimd.dma_gather(
                    y_w[:].rearrange("p (t e) -> p t e", e=256)[:, 0:t_a, :],
                    ltabA[:],
                    meta_w[:, oA:oA + t_a * 8],
                    t_a * 128, t_a * 128, 256, single_packet=False)
                nc.gpsimd.dma_gather(
                    y_w[:].rearrange("p (t e) -> p t e", e=256)[:, t_a:tt, :],
                    ltabB[:],
                    meta_w[:, oB:oB + t_b * 8],
                    t_b * 128, t_b * 128, 256, single_packet=False)
                emb_w = wpool.tile([10, tt * 128], bf16, tag="emb")
                nc.sync.dma_start(out=emb_w[:], in_=embT[:, w * ew:(w + 1) * ew])
                xw_a = wpool.tile([128, 128], bf16, tag="xwa")
                nc.sync.dma_start(out=xw_a[:], in_=xwin[0:128, w * 128:(w + 1) * 128])
                xw_b = wpool.tile([32, 128], bf16, tag="xwb")
                nc.sync.dma_start(out=xw_b[:], in_=xwin[128:160, w * 128:(w + 1) * 128])

                # radial MLP layer 1 (bf16), relu -> bf16
                hT = spool.tile([100, tt * 128], bf16, tag="hT", bufs=3)
                for c5 in range(n5):
                    ne = min(512, tt * 128 - c5 * 512)
                    ph = psum.tile([100, 512], f32, tag="ph", bufs=3)
                    nc.tensor.matmul(out=ph[:, :ne],
                                     lhsT=wfc1_sb[:],
                                     rhs=emb_w[:, c5 * 512:c5 * 512 + ne],
                                     start=True, stop=True)
                    nc.scalar.activation(hT[:, c5 * 512:c5 * 512 + ne], ph[:, :ne], relu)

                # radial layer 2 (bf16), two edge tiles per PSUM bank
                w_w = spool.tile([128, tt * 192], bf16, tag="ww", bufs=3)
                for t2_ in range(0, tt, 2):
                    ntl = min(2, tt - t2_)
                    pw = psum.tile([128, 384], f32, tag="pw")
                    for b in range(ntl):
                        nc.tensor.matmul(out=pw[:, b * 192:(b + 1) * 192],
                                         lhsT=hT[:, (t2_ + b) * 128:(t2_ + b + 1) * 128],
                                         rhs=wfc2_sb[:], start=True, stop=True)
                    nc.scalar.copy(out=w_w[:, t2_ * 192:(t2_ + ntl) * 192],
                                   in_=pw[:, :ntl * 192])

                # selection matrices A (bf16 0/1), prebuilt on host
                A_w = wpool.tile([128, tt * 128], bf16, tag="A")
                nc.sync.dma_start(out=A_w[:],
                                  in_=Asel[:, w * tt * 128:(w + 1) * tt * 128])

                # messages M [128, tt, 384] bf16
                M_w = spool.tile([128, tt * 384], bf16, tag="M")
                y3 = y_w[:].rearrange("p (t e) -> p t e", e=256)
                w3 = w_w[:].rearrange("p (t e) -> p t e", e=192)
                m3 = M_w[:].rearrange("p (t e) -> p t e", e=384)
                ea3 = ea_w.rearrange("p (t e) -> p t e", e=4)

                def eb(col, n):
                    return ea3[:, :, col:col + 1].to_broadcast([128, tt, n])

                t0_s = spool.tile([128, tt * 64], bf16, tag="t0")
                t0v = t0_s[:].rearrange("p (t e) -> p t e", e=64)
                t1_s = spool.tile([128, tt * 64], bf16, tag="t1")
                t1v = t1_s[:].rearrange("p (t e) -> p t e", e=64)
                t2_s = spool.tile([128, tt * 32], bf16, tag="t2")
                t2v = t2_s[:].rearrange("p (t e) -> p t e", e=32)
                z_s = spool.tile([128, tt * 96], bf16, tag="z")
                zv = z_s[:].rearrange("p (t e) -> p t e", e=96)
                zz_s = spool.tile([128, tt * 32], bf16, tag="zz")
                zzv = zz_s[:].rearrange("p (t e) -> p t e", e=32)

                tt_ = nc.vector.tensor_tensor
                # m0 = (w0*y0)*e0
                tt_(out=t0v, in0=w3[:, :, 0:64], in1=y3[:, :, 0:64], op=mult)
                tt_(out=m3[:, :, 0:64], in0=t0v, in1=eb(0, 64), op=mult)
                # m1_d = (w1*y0)*e1d
                tt_(out=t1v, in0=w3[:, :, 64:128], in1=y3[:, :, 0:64], op=mult)
                for d in range(3):
                    tt_(out=m3[:, :, 64 + 64 * d:128 + 64 * d],
                        in0=t1v, in1=eb(1 + d, 64), op=mult)
                # m2_d = (w2*e0)*y1_d
                tt_(out=t2v, in0=w3[:, :, 128:160], in1=eb(0, 32), op=mult)
                for d in range(3):
                    tt_(out=m3[:, :, 256 + 32 * d:288 + 32 * d],
                        in0=t2v, in1=y3[:, :, 64 + 32 * d:96 + 32 * d], op=mult)
                # m3 = w3 * sum_d(y1_d*e1_d)
                for d in range(3):
                    tt_(out=zv[:, :, 32 * d:32 * (d + 1)],
                        in0=y3[:, :, 64 + 32 * d:96 + 32 * d], in1=eb(1 + d, 32), op=mult)
                tt_(out=zzv, in0=zv[:, :, 0:32], in1=zv[:, :, 32:64], op=addop)
                tt_(out=zzv, in0=zzv, in1=zv[:, :, 64:96], op=addop)
                tt_(out=m3[:, :, 352:384], in0=zzv, in1=w3[:, :, 160:192], op=mult)

                # segment-sum: sT[f, slot] += M_chunk.T @ A  (3 chunks, acc over t)
                pst = psum.tile([128, 384], f32, tag="pst")
                for ch in range(3):
                    for t in range(tt):
                        nc.tensor.matmul(
                            out=pst[:, ch * 128:(ch + 1) * 128],
                            lhsT=m3[:, t, ch * 128:(ch + 1) * 128],
                            rhs=A_w[:, t * 128:(t + 1) * 128],
                            start=(t == 0), stop=(t == tt - 1))
                sT_sb = spool.tile([128, 384], bf16, tag="sT")
                nc.scalar.copy(out=sT_sb[:], in_=pst[:])

                # fused lin2 + self-interaction: out[slot, 0:160], all bf16
                po = psum.tile([128, 160], f32, tag="po", bufs=1)
                for ch in range(3):
                    nc.tensor.matmul(out=po[:],
                                     lhsT=sT_sb[:, ch * 128:(ch + 1) * 128],
                                     rhs=wbig_sb[:, ch * 160:(ch + 1) * 160],
                                     start=(ch == 0), stop=False)
                nc.tensor.matmul(out=po[:], lhsT=xw_a[:],
                                 rhs=wbig_sb[:, 480:640],
                                 start=False, stop=False)
                nc.tensor.matmul(out=po[:], lhsT=xw_b[:],
                                 rhs=wbig_sb[0:32, 640:800],
                                 start=False, stop=True)
                o_sb = spool.tile([128, 160], f32, tag="o")
                nc.scalar.copy(out=o_sb[:], in_=po[:])
                nc.sync.dma_start(out=out[w * 128:(w + 1) * 128, :], in_=o_sb[:])
            ps_ctx.__exit__(None, None, None)

    nc.compile()
    return nc


# --------------------------------------------------------------------------
# Host-side preparation
# --------------------------------------------------------------------------

def prepare(inputs, n_nodes=N_NODES, num_cores=N_CORES, split=SPLIT):
    npc = n_nodes // num_cores
    n_win = (npc + WIN - 1) // WIN

    f32 = np.float32
    node_input = np.asarray(inputs["node_input"], f32)
    node_attr = np.asarray(inputs["node_attr"], f32)
    edge_attr = np.asarray(inputs["edge_attr"], f32)
    emb = np.asarray(inputs["edge_length_embedded"], f32)
    src = np.asarray(inputs["edge_src"], np.int64)
    dst = np.asarray(inputs["edge_dst"], np.int64)
    E = src.shape[0]

    # fold node_attr into node features; de-interleave x1 by d
    xa = node_input * node_attr
    xg = np.concatenate([xa[:, :MUL0], xa[:, MUL0 + 0::3],
                         xa[:, MUL0 + 1::3], xa[:, MUL0 + 2::3]], axis=1)
    xaT = np.ascontiguousarray(xg.T).astype(BF16)         # [160, n_nodes]

    # fold node_attr[dst] into edge_attr
    eattr_f = edge_attr * node_attr[dst, 0][:, None]

    # weights with norm constants folded; phase-1 block-diagonal layout
    Wl10 = np.asarray(inputs["W_l1_0"], f32) / np.sqrt(MUL0).astype(f32)
    Wl11 = np.asarray(inputs["W_l1_1"], f32) / np.sqrt(MUL1).astype(f32)
    Wbd = np.zeros((128, 160), f32)
    Wbd[0:64, 0:64] = Wl10
    Wbd[64:96, 64:96] = Wl11
    Wbd[96:128, 96:128] = Wl11
    W2b = np.zeros((32, 160), f32)
    W2b[:, 128:160] = Wl11
    Wfc1 = (np.asarray(inputs["W_fc1"], f32) / np.sqrt(np.float32(N_BASIS))).astype(BF16)
    Wfc2b = (np.asarray(inputs["W_fc2"], f32) * (RELU_GAIN / np.sqrt(np.float32(N_RADIAL)))).astype(BF16)

    c2 = np.float32(0.5 / np.sqrt(NUM_NEIGHBORS) / FAN_L2)
    W2cat = np.zeros((384, 160), f32)
    W2cat[0:64, 0:64] = np.asarray(inputs["W_l2_00"], f32) * c2
    W2cat[352:384, 0:64] = np.asarray(inputs["W_l2_10"], f32) * c2 * INV_SQRT3
    for d in range(3):
        W2cat[64 + 64 * d:128 + 64 * d, 64 + 32 * d:96 + 32 * d] = \
            np.asarray(inputs["W_l2_01"], f32) * c2
        W2cat[256 + 32 * d:288 + 32 * d, 64 + 32 * d:96 + 32 * d] = \
            np.asarray(inputs["W_l2_11"], f32) * c2
    Wsi = np.zeros((160, 160), f32)
    Wsi[0:64, 0:64] = np.asarray(inputs["W_si0"], f32) / np.sqrt(MUL0).astype(f32)
    for d in range(3):
        Wsi[64 + 32 * d:96 + 32 * d, 64 + 32 * d:96 + 32 * d] = \
            np.asarray(inputs["W_si1"], f32) / np.sqrt(MUL1).astype(f32)
    Wfull = np.vstack([W2cat, Wsi])                       # [544, 160]
    Wbig = np.zeros((128, 5 * 160), f32)
    for ch in range(4):
        Wbig[:, ch * 160:(ch + 1) * 160] = Wfull[ch * 128:(ch + 1) * 128]
    Wbig[0:32, 640:800] = Wfull[512:544]

    iota = np.broadcast_to(np.arange(128, dtype=f32), (128, 128)).astype(BF16)

    # ---- dst-node -> (core, window, slot) assignment, degree-balanced:
    # round-robin of degree-sorted nodes over the core*win bins minimizes
    # the max per-window edge count (and so the padded tile counts).
    isA_e = src < split
    deg_A = np.bincount(dst[isA_e], minlength=n_nodes)
    deg_B = np.bincount(dst[~isA_e], minlength=n_nodes)
    ordern = np.argsort(-(deg_A * 1000 + deg_B), kind="stable")
    nbins = num_cores * n_win
    binid = np.empty(n_nodes, np.int64)
    slotid = np.empty(n_nodes, np.int64)
    for r in range(WIN):
        seg = ordern[r * nbins:(r + 1) * nbins]
        if len(seg) == 0:
            break
        b = np.arange(len(seg))
        binid[seg] = b if r % 2 == 0 else (nbins - 1 - b)
        slotid[seg] = r

    core = binid[dst] // n_win
    win = binid[dst] % n_win
    slot = slotid[dst]
    isA = isA_e.astype(np.int64)
    nk = num_cores * n_win * 2
    key = (core * n_win + win) * 2 + (1 - isA)
    order = np.argsort(key, kind="stable")
    sk = key[order]
    cnt = np.bincount(key, minlength=nk)
    cntA = cnt[0::2].reshape(num_cores, n_win)
    cntB = cnt[1::2].reshape(num_cores, n_win)
    t_a = max(1, int(-(-cntA.max() // 128)))
    t_b = max(1, int(-(-cntB.max() // 128)))
    tt = t_a + t_b
    ew = tt * 128
    e_core = n_win * ew
    mcols = ((4 + tt * 13) + 1) // 2 * 2
    assert cntA.min() > 0 and cntB.min() > 0

    grp_start = np.searchsorted(sk, np.arange(nk))
    pos = np.arange(E) - grp_start[sk]
    c_s = sk // (n_win * 2)
    w_s = (sk // 2) % n_win
    b_s = sk % 2
    dstpos = c_s * e_core + w_s * ew + b_s * (t_a * 128) + pos

    perm = np.full(num_cores * e_core, -1, np.int64)
    perm[dstpos] = order
    valid = perm >= 0
    pidx = np.where(valid, perm, 0)

    emb_p = (emb[pidx] * valid[:, None]).astype(BF16)     # [8EC, 10]
    ea_p = (eattr_f[pidx] * valid[:, None]).astype(BF16)  # [8EC, 4]
    blockpat = np.concatenate([np.zeros(t_a * 128, np.int64),
                               np.ones(t_b * 128, np.int64)])
    blockpat = np.tile(blockpat, num_cores * n_win)
    iv = np.where(valid, src[pidx] - split * blockpat, 0).astype(np.int16)
    sl_p = np.where(valid, slot[pidx], 0).astype(BF16)

    # selection matrices (slot one-hot per edge; zero rows for padding)
    slotv = np.where(valid, slot[pidx], -1)
    A_p = (slotv[:, None] == np.arange(128)[None, :]).astype(BF16)
    A_c = A_p.reshape(num_cores, n_win, tt, 128, 128) \
        .transpose(0, 3, 1, 2, 4).reshape(num_cores, 128, n_win * tt * 128)


    # device layouts
    embT_c = emb_p.reshape(num_cores, e_core, 10).transpose(0, 2, 1).copy()
    ea_c = ea_p.reshape(num_cores, n_win, tt, 128, 4).transpose(0, 3, 1, 2, 4) \
        .reshape(num_cores, 128, n_win, tt * 4).view(np.int16)
    dl_c = sl_p.reshape(num_cores, n_win, tt, 128).transpose(0, 3, 1, 2) \
        .reshape(num_cores, 128, n_win, tt).view(np.int16)
    ivr = iv.reshape(num_cores, n_win, tt * 128)
    ivA = ivr[:, :, :t_a * 128].reshape(num_cores, n_win, t_a * 8, 16)
    idxA_c = np.tile(ivA.transpose(0, 3, 1, 2).reshape(num_cores, 16, n_win, t_a * 8),
                     (1, 8, 1, 1))
    ivB = ivr[:, :, t_a * 128:].reshape(num_cores, n_win, t_b * 8, 16)
    idxB_c = np.tile(ivB.transpose(0, 3, 1, 2).reshape(num_cores, 16, n_win, t_b * 8),
                     (1, 8, 1, 1))

    cnts = np.stack([cntA, cntB], axis=-1).astype(np.int32)  # [cores, n_win, 2]
    cnts_i16 = cnts.view(np.int16).reshape(num_cores, 1, n_win, 4)

    meta_c = np.zeros((num_cores, 128, n_win, mcols), np.int16)
    meta_c[:, :, :, 0:4] = cnts_i16
    meta_c[:, :, :, 4:4 + tt * 4] = ea_c
    meta_c[:, :, :, 4 + tt * 4:4 + tt * 5] = dl_c
    meta_c[:, :, :, 4 + tt * 5:4 + tt * 5 + t_a * 8] = idxA_c
    meta_c[:, :, :, 4 + tt * 5 + t_a * 8:4 + tt * 13] = idxB_c
    meta_c = meta_c.reshape(num_cores, 128, n_win * mcols)

    nodes_at = np.zeros(nbins * WIN, np.int64)
    nodes_at[binid * WIN + slotid] = np.arange(n_nodes)
    occ = np.zeros(nbins * WIN, bool)
    occ[binid * WIN + slotid] = True
    xwin_all = xaT[:, nodes_at].copy()
    xwin_all[:, ~occ] = 0
    xwin_c = xwin_all.reshape(160, num_cores, n_win * 128).transpose(1, 0, 2)


    in_maps = []
    for c in range(num_cores):
        in_maps.append({
            "xaT": xaT, "xwin": xwin_c[c],
            "embT": embT_c[c], "meta": meta_c[c], "Asel": A_c[c],
            "Wbd": Wbd.astype(BF16), "W2b": W2b.astype(BF16),
            "Wfc1": Wfc1, "Wfc2b": Wfc2b,
            "Wbig": Wbig.astype(BF16),
        })
    meta = dict(n_nodes=n_nodes, npc=npc, n_win=n_win, t_a=t_a, t_b=t_b,
                num_cores=num_cores, split=split,
                binid=binid, slotid=slotid)
    return in_maps, meta


def assemble(results, meta):
    """results: list of per-core dicts with 'out' [n_win*128,160] deint."""
    fullb = np.concatenate([r["out"] for r in results], axis=0)
    full = fullb[meta["binid"] * WIN + meta["slotid"]]
    out = np.empty_like(full)
    out[:, :MUL0] = full[:, :MUL0]
    for d in range(3):
        out[:, MUL0 + d::3] = full[:, MUL0 + 32 * d:MUL0 + 32 * (d + 1)]
    return np.ascontiguousarray(out, dtype=np.float32)


_LAST_NC = None
_LAST_INMAPS = None
_LAST_META = None


def kernel(**inputs):
    global _LAST_RESULTS, _LAST_NC, _LAST_INMAPS, _LAST_META
    in_maps, meta = prepare(inputs)
    nc = build_program(meta["n_nodes"], meta["npc"], meta["n_win"],
                       meta["t_a"], meta["t_b"], meta["num_cores"],
                       split=meta["split"])
    _LAST_NC, _LAST_INMAPS, _LAST_META = nc, in_maps, meta
    res = bass_utils.run_bass_kernel_spmd(
        nc, in_maps, core_ids=list(range(meta["num_cores"])))
    _LAST_RESULTS = res
    return assemble(res.results, meta)
